# revision 13
# baseline (speedup 1.0000x reference)
"""Causal self-attention TRN2 kernel: build + host glue. (v6.2)

Sharding: tensor-parallel over heads. 16 heads / 8 cores = 2 heads per core.
Each core computes q/k/v for its 2 heads over all 4x2048 tokens, runs causal
attention, and produces a partial output projection outT [1024, 8192] (fp16)
(wp rows for its heads only). Host sums the 8 partials and transposes.

v6 changes over v5 (322us):
- QKV projections in fp8 (e4m3) with DoubleRow perf mode: 256-deep
  contraction per pass -> 4 matmuls per projection per token tile (was 8).
  Host scales W by 32 so fp8 weights have unit variance; the 1/32 is folded
  into the rope cos/sin tables (q,k) and into W_proj (v path).
- AV matmul in fp8 DoubleRow over key-tile PAIRS (256 keys per pass): exp
  output is written as fp8e4m3 (bias -2.25 inside the exp keeps values in
  [0, ~90] clear of e4m3 saturation at 240; the uniform e^-2.25 cancels in
  the softmax normalize). The ones-augmented V columns still produce the
  softmax denominators for free in PSUM rows 64-127.
- Causal masking moved off DVE onto the PE: one accumulation matmul
  (-1e4 * I stationary, precomputed [128,2,256] mask pattern moving) adds
  -1e4 to all masked score entries before the EXP.
- Early tokens (first 128 queries of each batch) have concentrated softmax
  weights, so fp8 elementwise noise passes through unsuppressed and blows
  the max-err metric. A small full-fp16 sidecar recomputes those 4x128
  queries exactly (fp16 QKV + rope + scores + AV for keys 0-127) and
  overwrites yTn[:, 0:128] before the output projection.
- AV tail + softmax normalize of each q-tile are deferred into the next
  q-tile's filler stream so the PE never waits on the final EXPs at a
  q-tile boundary (HAM clock-gate oscillation was costing ~45us at half
  clock). Warm matmuls are interleaved into the kernel tail for the same
  reason.
"""

from collections import deque
from contextlib import ExitStack

import numpy as np

import concourse.bacc as bacc
import concourse.bass as bass
import concourse.mybir as mybir
import concourse.tile as tile

F32 = mybir.dt.float32
FP16 = mybir.dt.float16
FP8 = mybir.dt.float8e4
AF = mybir.ActivationFunctionType
DR = mybir.MatmulPerfMode.DoubleRow

D = 1024
H = 16
DH = 64
S = 2048
B = 4
NCORE = 8
HPC = 2  # heads per core
NT = S // 512  # 4 token tiles per batch
NKT = S // 128  # 16 key tiles per batch
NPAIR = NKT // 2  # 8 key-tile pairs per batch

SWAP_MASK = [(i + 16) % 32 for i in range(32)]
EXP_BIAS = -2.25
WSCALE = 32.0
import os
DEFER_TAIL = os.environ.get("K_DEFER_TAIL", "1") == "1"
TAIL_WARM = os.environ.get("K_TAIL_WARM", "1") == "1"
SIDECAR = os.environ.get("K_SIDECAR", "1") == "1"
MERGED_TP = os.environ.get("K_MERGED_TP", "1") == "1"


def build(nb=B):
    nc = bacc.Bacc("TRN2")
    xT = nc.dram_tensor("xT", [D, B * S], FP8, kind="ExternalInput")
    wq = nc.dram_tensor("wq", [D, 128], FP8, kind="ExternalInput")
    wk = nc.dram_tensor("wk", [D, 128], FP8, kind="ExternalInput")
    wv = nc.dram_tensor("wv", [D, 128], FP8, kind="ExternalInput")
    wq16 = nc.dram_tensor("wq16", [D, 128], FP16, kind="ExternalInput")
    wk16 = nc.dram_tensor("wk16", [D, 128], FP16, kind="ExternalInput")
    wv16 = nc.dram_tensor("wv16", [D, 128], FP16, kind="ExternalInput")
    x16h = nc.dram_tensor("x16h", [D, B * 128], FP16, kind="ExternalInput")
    wp = nc.dram_tensor("wp", [128, D], FP16, kind="ExternalInput")
    cs1 = nc.dram_tensor("cs1", [128, S], FP16, kind="ExternalInput")
    cs2 = nc.dram_tensor("cs2", [128, S], FP16, kind="ExternalInput")
    cs1u = nc.dram_tensor("cs1u", [128, B * 128], FP16, kind="ExternalInput")
    cs2u = nc.dram_tensor("cs2u", [128, B * 128], FP16, kind="ExternalInput")
    maskC = nc.dram_tensor("maskC", [128, 512], FP16, kind="ExternalInput")
    negI = nc.dram_tensor("negI", [128, 128], FP16, kind="ExternalInput")
    ident = nc.dram_tensor("ident", [128, 128], FP16, kind="ExternalInput")
    outT = nc.dram_tensor("outT", [D, B * S], FP16, kind="ExternalOutput")
    warm = nc.dram_tensor("warm", [128, 128], FP16, kind="ExternalOutput")

    with tile.TileContext(nc) as tc, ExitStack() as ctx, nc.allow_low_precision(
        reason="fp8 matmul operands with fp32 accumulation; adequate accuracy"
    ):
        const = ctx.enter_context(tc.tile_pool(name="const", bufs=1))
        xt_pool = ctx.enter_context(tc.tile_pool(name="xt", bufs=3))
        slab = ctx.enter_context(tc.tile_pool(name="slab", bufs=3))
        tmp_pool = ctx.enter_context(tc.tile_pool(name="tmp", bufs=3))
        ex_pool = ctx.enter_context(tc.tile_pool(name="ex", bufs=5))
        ytn_pool = ctx.enter_context(tc.tile_pool(name="ytn", bufs=3))
        ob_pool = ctx.enter_context(tc.tile_pool(name="ob", bufs=4))
        # PSUM (8 banks): sc 2x[128,2,512]f32=4, yt 2x[128,512]f32=2, qp 2x=2
        ps_sc = ctx.enter_context(tc.tile_pool(name="ps_sc", bufs=2, space="PSUM"))
        ps_qp = ctx.enter_context(tc.tile_pool(name="ps_qp", bufs=2, space="PSUM"))
        ps_yt = ctx.enter_context(tc.tile_pool(name="ps_yt", bufs=2, space="PSUM"))

        # ---- constants (ordered so the first tile's work can start asap) ----
        id_sb = const.tile([128, 128], FP16)
        nc.sync.dma_start(out=id_sb[:], in_=ident[:])
        wq_sb = const.tile([128, 8, 128], FP8)
        nc.sync.dma_start(
            out=wq_sb[:, 0:4, :], in_=wq[:].rearrange("(a p) c -> p a c", p=128)[:, 0:4]
        )
        nc.sync.dma_start(
            out=wq_sb[:, 4:8, :], in_=wq[:].rearrange("(a p) c -> p a c", p=128)[:, 4:8]
        )
        xt0 = xt_pool.tile([128, 8, 512], FP8, tag="xt", name="xt0")
        for dt in range(8):
            nc.sync.dma_start(out=xt0[:, dt, :], in_=xT[bass.ts(dt, 128), 0:512])
        wk_sb = const.tile([128, 8, 128], FP8)
        wv_sb = const.tile([128, 8, 128], FP8)
        for w_sb, w_dram in ((wk_sb, wk), (wv_sb, wv)):
            nc.sync.dma_start(
                out=w_sb[:, 0:4, :],
                in_=w_dram[:].rearrange("(a p) c -> p a c", p=128)[:, 0:4],
            )
            nc.sync.dma_start(
                out=w_sb[:, 4:8, :],
                in_=w_dram[:].rearrange("(a p) c -> p a c", p=128)[:, 4:8],
            )
        cs1_sb = const.tile([128, S], FP16)
        cs2_sb = const.tile([128, S], FP16)
        nc.sync.dma_start(out=cs1_sb[:, 0:1024], in_=cs1[:, 0:1024])
        nc.sync.dma_start(out=cs2_sb[:, 0:1024], in_=cs2[:, 0:1024])
        mask_sb = const.tile([128, 2, 256], FP16)
        nc.sync.dma_start(
            out=mask_sb[:], in_=maskC[:].rearrange("p (a c) -> p a c", c=256)
        )
        negI_sb = const.tile([128, 128], FP16)
        nc.sync.dma_start(out=negI_sb[:], in_=negI[:])
        wp_sb = const.tile([128, D], FP16)
        nc.sync.dma_start(out=wp_sb[:], in_=wp[:])
        nc.sync.dma_start(out=cs1_sb[:, 1024:2048], in_=cs1[:, 1024:2048])
        nc.sync.dma_start(out=cs2_sb[:, 1024:2048], in_=cs2[:, 1024:2048])
        # fp16 sidecar constants (first 128 tokens of each batch)
        w16_sb = {}
        for nm, w_dram in (("q", wq16), ("k", wk16), ("v", wv16)):
            w16 = const.tile([128, 8, 128], FP16)
            nc.sync.dma_start(
                out=w16[:], in_=w_dram[:].rearrange("(a p) c -> p a c", p=128)
            )
            w16_sb[nm] = w16
        x16_sb = const.tile([128, 8, 512], FP16)
        for dt in range(8):
            nc.sync.dma_start(out=x16_sb[:, dt, :], in_=x16h[bass.ts(dt, 128), :])
        cs1u_sb = const.tile([128, 512], FP16)
        cs2u_sb = const.tile([128, 512], FP16)
        nc.sync.dma_start(out=cs1u_sb[:], in_=cs1u[:])
        nc.sync.dma_start(out=cs2u_sb[:], in_=cs2u[:])

        bias_sb = const.tile([128, 1], F32)
        nc.vector.memset(bias_sb[:], EXP_BIAS)
        # precise yTn for queries 0-127 of each batch, written by the
        # sidecar, copied over yTn at each batch's j=0 normalize
        ytn0 = const.tile([128, B * 128], FP16)
        v0aug = const.tile([128, B, HPC, 128], FP16)
        nc.vector.memset(v0aug[:, :, :, 64:128], 1.0)

        warm_ps = ps_qp.tile([128, 128], F32, tag="qp", name="warm_ps")
        for i in range(40):
            nc.tensor.matmul(warm_ps[:], id_sb[:], id_sb[:],
                             start=True, stop=True)
        warm_sb = tmp_pool.tile([128, 128], FP16, tag="t1", name="warm_sb")
        nc.vector.tensor_copy(warm_sb[:], warm_ps[:])
        nc.sync.dma_start(out=warm[:], in_=warm_sb[:])

        # pre-initialize the ones columns of all three v_sb rotation buffers
        # (only cols 0:64 of each [128,128] block are rewritten per batch)
        vsb_bufs = []
        for i in range(3):
            v_sb = slab.tile([128, HPC * NKT * 128], FP8, tag="v_sb", name="v_sb")
            v3d = v_sb[:].rearrange("p (n c) -> p n c", c=128)
            nc.vector.memset(v3d[:, 0 : HPC * NKT, 64:128], 1.0)
            vsb_bufs.append(v_sb)

        def rope16(dst_slice, qs, cs1_t, cs2_t, w=512):
            """dst(fp16) = qs*cs1_t + shuffle(qs)*cs2_t ; qs is [128,w] fp16."""
            sw = tmp_pool.tile([128, w], FP16, tag="sw", name="sw")
            nc.vector.stream_shuffle(sw[:], qs[:], mask=SWAP_MASK)
            t1 = tmp_pool.tile([128, w], FP16, tag="t1", name="t1")
            nc.vector.tensor_mul(t1[:], qs[:], cs1_t)
            t2 = tmp_pool.tile([128, w], FP16, tag="t2", name="t2")
            nc.vector.tensor_mul(t2[:], sw[:], cs2_t)
            nc.vector.tensor_add(dst_slice, t1[:], t2[:])

        # ---- fp16 sidecar: exact attention for queries 0-127 of each batch
        # (keys 0-127 only, causal). Emitted as early filler; writes ytn0.
        def sidecar_chunks():
            st8 = {}

            def mm8(ps, w16, n0, n1):
                for i in range(8):
                    nc.tensor.matmul(
                        ps[:], w16[:, i, :], x16_sb[:, i, :],
                        start=i == 0, stop=i == 7,
                    )

            def s_q():
                q_ps = ps_qp.tile([128, 512], F32, tag="qp", name="q0_ps")
                mm8(q_ps, w16_sb["q"], 0, 8)
                st8["q_ps"] = q_ps

            def s_k():
                q0s = tmp_pool.tile([128, 512], FP16, tag="qs", name="q0s")
                nc.scalar.copy(q0s[:], st8["q_ps"][:])
                st8["q0s"] = q0s
                k_ps = ps_qp.tile([128, 512], F32, tag="qp", name="k0_ps")
                mm8(k_ps, w16_sb["k"], 0, 8)
                st8["k_ps"] = k_ps

            def s_v():
                k0s = tmp_pool.tile([128, 512], FP16, tag="ks", name="k0s")
                nc.scalar.copy(k0s[:], st8["k_ps"][:])
                st8["k0s"] = k0s
                q0r = const.tile([128, 512], FP16)
                rope16(q0r[:], st8["q0s"], cs1u_sb[:], cs2u_sb[:])
                st8["q0r"] = q0r
                v_ps = ps_qp.tile([128, 512], F32, tag="qp", name="v0_ps")
                mm8(v_ps, w16_sb["v"], 0, 8)
                st8["v_ps"] = v_ps

            def s_tp():
                v0st = tmp_pool.tile([128, 512], FP16, tag="vst", name="v0st")
                nc.scalar.copy(v0st[:], st8["v_ps"][:])
                k0r = const.tile([128, 512], FP16)
                rope16(k0r[:], st8["k0s"], cs1u_sb[:], cs2u_sb[:])
                st8["k0r"] = k0r
                v0T = ps_qp.tile([128, 512], FP16, tag="qp", name="v0T")
                for c in range(4):
                    nc.tensor.transpose(
                        v0T[:, bass.ts(c, 128)],
                        v0st[:, bass.ts(c, 128)],
                        id_sb[:],
                    )
                v4 = v0T[:].rearrange("p (c hd) -> p c hd", hd=128)
                for h in range(HPC):
                    nc.vector.tensor_copy(
                        v0aug[:, :, h, 0:64],
                        v4[:, :, h * 64 : h * 64 + 64],
                    )

            def s_att(c):
                def run():
                    # NB: one PSUM bank cannot host concurrent accumulation
                    # groups with different PE tile positions (head A rows
                    # 0-63 vs head B rows 64-127) -> per-head tiles.
                    q0r, k0r = st8["q0r"], st8["k0r"]
                    sch = {}
                    for h in range(HPC):
                        sch[h] = ps_qp.tile([128, 512], F32, tag="qp",
                                            name=f"sc0{h}")
                    for h in range(HPC):
                        nc.tensor.matmul(
                            sch[h][:, 0:128],
                            k0r[bass.ts(h, 64), bass.ts(c, 128)],
                            q0r[bass.ts(h, 64), bass.ts(c, 128)],
                            start=True, stop=True,
                        )
                    for h in range(HPC):
                        nc.tensor.matmul(
                            sch[h][:, 0:128], negI_sb[:], mask_sb[:, 0, 0:128],
                            start=False, stop=True, skip_group_check=True,
                        )
                    ex0 = tmp_pool.tile([128, 2, 128], FP16, tag="ex0", name="ex0")
                    for h in range(HPC):
                        nc.scalar.activation(
                            ex0[:, h, :], sch[h][:, 0:128], AF.Exp,
                            scale=0.125, bias=bias_sb[:],
                        )
                    for h in range(HPC):
                        yt0 = ps_qp.tile([128, 512], F32, tag="qp", name="yt0")
                        nc.tensor.matmul(
                            yt0[:, 0:128], v0aug[:, c, h, :], ex0[:, h, :],
                            start=True, stop=True,
                        )
                        den = tmp_pool.tile([64, 128], F32, tag="den", name="den0")
                        nc.scalar.copy(den[:], yt0[64:128, 0:128])
                        rc = tmp_pool.tile([64, 128], F32, tag="rc64", name="rc0")
                        nc.vector.reciprocal_approx_fast(rc[:], den[:])
                        nc.vector.tensor_mul(
                            ytn0[bass.ts(h, 64), bass.ts(c, 128)],
                            yt0[0:64, 0:128],
                            rc[:],
                        )

                return run

            return [s_q, s_k, s_v, s_tp] + [s_att(c) for c in range(4)]

        # batch-generation state (slab tiles rotate per batch)
        cur = {}

        def p1_chunks(b, t, prefetch):
            """Return list of emission closures for token tile t of batch b.

            prefetch: (b', t') of the NEXT token tile, or None; its x DMA is
            emitted inside the first chunk.
            """

            def c_start():
                if t == 0:
                    qT = slab.tile([128, S], FP16, tag="qT", name="qT")
                    kT = slab.tile([128, S], FP16, tag="kT", name="kT")
                    v_sb = vsb_bufs[b % 3]
                    cur[b] = (qT, kT, v_sb)
                if prefetch is not None:
                    pb, pt = prefetch
                    xt_n = xt_pool.tile([128, 8, 512], FP8, tag="xt", name="xt_n")
                    for dt in range(8):
                        nc.sync.dma_start(
                            out=xt_n[:, dt, :],
                            in_=xT[
                                bass.ts(dt, 128),
                                pb * S + pt * 512 : pb * S + (pt + 1) * 512,
                            ],
                        )
                    cur["xt", pb, pt] = xt_n

            state = {}

            def get_xt():
                return cur.pop(("xt", b, t)) if ("xt", b, t) in cur else xt0

            def dr_proj(ps, w_sb, x):
                for i in range(4):
                    nc.tensor.matmul(
                        ps[:], w_sb[:, 2 * i : 2 * i + 2, :],
                        x[:, 2 * i : 2 * i + 2, :],
                        start=i == 0, stop=i == 3, perf_mode=DR,
                    )

            def c_q():
                x = state.setdefault("xt", get_xt())
                q_ps = ps_qp.tile([128, 512], F32, tag="qp", name="q_ps")
                state["q_ps"] = q_ps
                dr_proj(q_ps, wq_sb, x)

            def c_k():
                qs = tmp_pool.tile([128, 512], FP16, tag="qs", name="qs")
                nc.scalar.copy(qs[:], state["q_ps"][:])
                state["qs"] = qs
                x = state["xt"]
                k_ps = ps_qp.tile([128, 512], F32, tag="qp", name="k_ps")
                state["k_ps"] = k_ps
                dr_proj(k_ps, wk_sb, x)

            def c_v():
                ks = tmp_pool.tile([128, 512], FP16, tag="ks", name="ks")
                nc.scalar.copy(ks[:], state["k_ps"][:])
                state["ks"] = ks
                rope16(
                    cur[b][0][:, bass.ts(t, 512)], state["qs"],
                    cs1_sb[:, bass.ts(t, 512)], cs2_sb[:, bass.ts(t, 512)],
                )
                x = state["xt"]
                v_ps = ps_qp.tile([128, 512], F32, tag="qp", name="v_ps")
                state["v_ps"] = v_ps
                dr_proj(v_ps, wv_sb, x)

            def c_vev():
                vstage = tmp_pool.tile([128, 512], FP16, tag="vst", name="vstage")
                nc.scalar.copy(vstage[:], state["v_ps"][:])
                state["vst"] = vstage
                rope16(
                    cur[b][1][:, bass.ts(t, 512)], state["ks"],
                    cs1_sb[:, bass.ts(t, 512)], cs2_sb[:, bass.ts(t, 512)],
                )

            def c_tp():
                # transpose both heads at once: [128,128] blocks of vstage
                # -> [tok, 2*64 dims]; scatter per head into v_sb (fp8)
                vstage = state["vst"]
                tpb = ps_qp.tile([128, 512], FP16, tag="qp", name="tpb")
                for kk in range(4):
                    nc.tensor.transpose(
                        tpb[:, bass.ts(kk, 128)],
                        vstage[:, bass.ts(kk, 128)],
                        id_sb[:],
                    )
                v_sb = cur[b][2]
                t4 = tpb[:].rearrange("p (n hd) -> p n hd", hd=128)
                for h in range(HPC):
                    dst = v_sb[:].rearrange("p (n c) -> p n c", c=128)[
                        :, h * NKT + t * 4 : h * NKT + t * 4 + 4, 0:64
                    ]
                    nc.vector.tensor_copy(dst, t4[:, :, h * 64 : h * 64 + 64])

            def c_tp_v5(h):
                def run():
                    vstage = state["vst"]
                    tp4 = ps_qp.tile([128, 256], FP16, tag="qp", name="tp4")
                    for kk in range(4):
                        nc.tensor.transpose(
                            tp4[:, bass.ts(kk, 64)],
                            vstage[bass.ts(h, 64), bass.ts(kk, 128)],
                            id_sb[bass.ts(h, 64), bass.ts(h, 64)],
                        )
                    v_sb = cur[b][2]
                    dst = v_sb[:].rearrange("p (n c) -> p n c", c=128)[
                        :, h * NKT + t * 4 : h * NKT + t * 4 + 4, 0:64
                    ]
                    nc.vector.tensor_copy(
                        dst, tp4[:].rearrange("p (n c) -> p n c", c=64)
                    )

                return run

            if MERGED_TP:
                return [c_start, c_q, c_k, c_v, c_vev, c_tp]
            return [c_start, c_q, c_k, c_v, c_vev, c_tp_v5(0), c_tp_v5(1)]

        def att_emit(b, j, fill, tail=False):
            """Emit attention for qtile j of batch b, draining `fill` units
            (independent PE work) evenly across the key-pair slots. The AV
            tail, softmax normalize, and output projection are returned as
            filler for the NEXT q-tile."""
            qT, kT, v_sb = cur[b]
            tok0 = b * S
            npair = 2 * (j + 1)
            n_fill = len(fill)
            popped = 0
            yts = {}
            for h in range(HPC):
                yts[h] = ps_yt.tile([128, 512], F32, tag="yt", name=f"yt{h}")
            exs = {}
            starts = {}
            v4d = v_sb[:].rearrange("p (n c) -> p n c", c=128)

            def av_p(p, last):
                st = starts[p]
                for h in range(HPC):
                    n0 = h * NKT + 2 * p
                    nc.tensor.matmul(
                        yts[h][:, st:512],
                        v4d[:, n0 : n0 + 2, :],
                        exs[p, h][:, :, st:512],
                        start=(p == 0),
                        stop=last,
                        perf_mode=DR,
                    )

            for p in range(npair):
                st = 256 if p == 2 * j + 1 else 0
                diag = p >= 2 * j
                starts[p] = st
                scs = {}
                for h in range(HPC):
                    # head A: PE rows 0-63, head B: rows 64-127 — emitted
                    # back-to-back so the K=64 row tiles run concurrently
                    scs[h] = ps_sc.tile([128, 2, 512], F32, tag="sc", name=f"sc{h}")
                for sl in range(2):
                    for h in range(HPC):
                        kt = 2 * p + sl
                        nc.tensor.matmul(
                            scs[h][:, sl, st:512],
                            kT[bass.ts(h, 64), bass.ts(kt, 128)],
                            qT[bass.ts(h, 64), j * 512 + st : (j + 1) * 512],
                            start=True,
                            stop=True,
                        )
                if diag:
                    # add -1e4 to masked (non-causal) entries of the two
                    # diagonal key tiles before the exp
                    for h in range(HPC):
                        nc.tensor.matmul(
                            scs[h][:, :, st : st + 256],
                            negI_sb[:],
                            mask_sb[:],
                            start=False,
                            stop=True,
                            skip_group_check=True,
                        )
                for h in range(HPC):
                    ex = ex_pool.tile([128, 2, 512], FP8, tag="ex", name="ex")
                    nc.scalar.activation(
                        ex[:, :, st:512], scs[h][:, :, st:512], AF.Exp,
                        scale=0.125, bias=bias_sb[:],
                    )
                    exs[p, h] = ex
                if p > 0:
                    av_p(p - 1, last=False)
                # drain filler units evenly
                want = (n_fill * (p + 1)) // (npair + 1)
                while popped < want and fill:
                    fill.popleft()()
                    popped += 1
            if not DEFER_TAIL:
                av_p(npair - 1, last=True)
            while fill and popped < n_fill:
                fill.popleft()()
                popped += 1

            yTn = ytn_pool.tile([128, 512], FP16, tag="ytn", name="yTn")

            def av_tail():
                if DEFER_TAIL:
                    av_p(npair - 1, last=True)

            def norm_chunk():
                for h in range(HPC):
                    # yt rows 64..127 hold the denominator (ones columns of
                    # v_aug); DVE drops partition offsets on PSUM reads, so
                    # the den rows are staged through SBUF on the scalar
                    # engine. Deferred into the next att's filler stream.
                    den = tmp_pool.tile([64, 512], F32, tag="den", name="den")
                    nc.scalar.copy(den[:], yts[h][64:128, :])
                    rc64 = tmp_pool.tile([64, 512], F32, tag="rc64", name="rc64")
                    nc.vector.reciprocal_approx_fast(rc64[:], den[:])
                    nc.vector.tensor_mul(
                        yTn[bass.ts(h, 64), :], yts[h][0:64, :], rc64[:]
                    )
                if j == 0 and SIDECAR:
                    # overwrite the concentrated-softmax early queries with
                    # the fp16 sidecar's exact values
                    nc.vector.tensor_copy(
                        yTn[:, 0:128], ytn0[:, bass.ts(b, 128)]
                    )

            def proj_chunk(dt):
                def run():
                    po = ps_qp.tile([128, 512], F32, tag="qp", name="po")
                    nc.tensor.matmul(
                        po[:], wp_sb[:, bass.ts(dt, 128)], yTn[:],
                        start=True, stop=True,
                    )
                    ob = ob_pool.tile([128, 512], FP16, tag="ob", name="ob")
                    nc.vector.tensor_copy(ob[:], po[:])
                    nc.sync.dma_start(
                        out=outT[
                            bass.ts(dt, 128), tok0 + j * 512 : tok0 + (j + 1) * 512
                        ],
                        in_=ob[:],
                    )
                    if tail and TAIL_WARM:
                        # keep the PE dense through the drain so the HAM
                        # clock gate stays at full rate
                        wp2 = ps_qp.tile([128, 512], F32, tag="qp", name="wm")
                        nc.tensor.matmul(
                            wp2[:], id_sb[:], cs1_sb[:, 0:512],
                            start=True, stop=True,
                        )

                return run

            return [av_tail, norm_chunk] + [proj_chunk(dt) for dt in range(8)]

        # ---- driver: p1 units run 2 steps ahead of att units ----
        p1s = [(b, t) for b in range(nb) for t in range(NT)]
        atts = [(b, j) for b in range(nb) for j in range(NT)]
        fill = deque()
        for i in range(len(p1s) + 2):
            if i < len(p1s):
                pref = p1s[i + 1] if i + 1 < len(p1s) else None
                fill.extend(p1_chunks(*p1s[i], prefetch=pref))
                if i == 0 and SIDECAR:
                    fill.extend(sidecar_chunks())
            if i >= 2:
                proj = att_emit(*atts[i - 2], fill=fill, tail=(i - 2) == len(atts) - 1)
                fill.extend(proj)
            elif i < 2:
                while fill:
                    fill.popleft()()
        while fill:
            fill.popleft()()
    nc.finalize()
    return nc


# ---------------- host side ----------------

def host_prepare(x, W_qkv, W_proj):
    import ml_dtypes

    fp8 = ml_dtypes.float8_e4m3

    def to_fp8(a):
        return np.clip(a, -240.0, 240.0).astype(fp8)

    xf = np.ascontiguousarray(np.asarray(x, dtype=np.float32).reshape(B * S, D))
    xT = np.ascontiguousarray(to_fp8(xf.T))
    # fp16 copy of the first 128 tokens of each batch for the sidecar
    x16h = np.ascontiguousarray(
        np.concatenate([xf[b * S : b * S + 128] for b in range(B)], axis=0).T
    ).astype(np.float16)
    Wq = np.asarray(W_qkv[:, 0:D], dtype=np.float32)
    Wk = np.asarray(W_qkv[:, D : 2 * D], dtype=np.float32)
    Wv = np.asarray(W_qkv[:, 2 * D : 3 * D], dtype=np.float32)
    Wp = np.asarray(W_proj, dtype=np.float32)
    half = DH // 2
    inv_freq = 1.0 / (10000.0 ** (np.arange(half, dtype=np.float64) / half))
    freqs = np.outer(np.arange(S, dtype=np.float64), inv_freq)  # [S, 32]
    cos = np.cos(freqs)
    sin = np.sin(freqs)
    # quadrant-local rope pair layout: per 32-slot quadrant q, slots 0-15
    # hold even dims of pairs 16q..16q+15, slots 16-31 the odd dims.
    perm = np.empty(DH, dtype=np.int64)
    cs1_h = np.empty((DH, S), dtype=np.float32)
    cs2_h = np.empty((DH, S), dtype=np.float32)
    for q in range(2):
        for i in range(32):
            k = 16 * q + (i % 16)
            r = 32 * q + i
            perm[r] = 2 * k if i < 16 else 2 * k + 1
            cs1_h[r] = cos[:, k]
            cs2_h[r] = -sin[:, k] if i < 16 else sin[:, k]
    # fold the 1/WSCALE compensation for the fp8 weight scaling into rope
    cs1 = (np.concatenate([cs1_h, cs1_h], axis=0) / WSCALE).astype(np.float16)
    cs2 = (np.concatenate([cs2_h, cs2_h], axis=0) / WSCALE).astype(np.float16)
    # unscaled rope tables for positions 0-127, repeated per batch
    cs1u = np.ascontiguousarray(
        np.tile(np.concatenate([cs1_h, cs1_h], axis=0)[:, 0:128], (1, B))
    ).astype(np.float16)
    cs2u = np.ascontiguousarray(
        np.tile(np.concatenate([cs2_h, cs2_h], axis=0)[:, 0:128], (1, B))
    ).astype(np.float16)
    ident = np.eye(128, dtype=np.float16)
    negI = (-1.0e4 * np.eye(128)).astype(np.float16)
    # mask pattern for a diagonal key-tile pair: moving operand [128, 2, 256]
    ii = np.arange(128)[:, None]
    jj = np.arange(128)[None, :]
    low = (ii > jj).astype(np.float16)  # strictly-lower = non-causal
    maskC = np.concatenate(
        [low, np.zeros((128, 128), np.float16),
         np.ones((128, 128), np.float16), low],
        axis=1,
    )
    in_maps = []
    for c in range(NCORE):
        hA, hB = HPC * c, HPC * c + 1

        def cols(W, h, p=None):
            w = W[:, h * DH : (h + 1) * DH]
            return w[:, p] if p is not None else w

        wq_c = np.concatenate([cols(Wq, hA, perm), cols(Wq, hB, perm)], axis=1)
        wk_c = np.concatenate([cols(Wk, hA, perm), cols(Wk, hB, perm)], axis=1)
        wv_c = np.concatenate([cols(Wv, hA), cols(Wv, hB)], axis=1)
        in_maps.append(
            {
                "xT": xT,
                "x16h": x16h,
                "wq": to_fp8(WSCALE * wq_c),
                "wk": to_fp8(WSCALE * wk_c),
                "wv": to_fp8(WSCALE * wv_c),
                "wq16": wq_c.astype(np.float16),
                "wk16": wk_c.astype(np.float16),
                "wv16": (wv_c * WSCALE).astype(np.float16),
                "wp": np.ascontiguousarray(
                    Wp[hA * DH : (hB + 1) * DH, :] / WSCALE
                ).astype(np.float16),
                "cs1": cs1,
                "cs2": cs2,
                "cs1u": cs1u,
                "cs2u": cs2u,
                "maskC": maskC,
                "negI": negI,
                "ident": ident,
            }
        )
    return in_maps


def kernel(x, W_qkv, W_proj):
    """Grading entrypoint: full inputs in, full output out.

    x [4, 2048, 1024] fp32, W_qkv [1024, 3072] fp32, W_proj [1024, 1024] fp32
    -> [4, 2048, 1024] fp32
    """
    from concourse.bass_utils import run_bass_kernel_spmd

    x = np.asarray(x)
    in_maps = host_prepare(x, np.asarray(W_qkv), np.asarray(W_proj))
    nc = build()
    res = run_bass_kernel_spmd(nc, in_maps, list(range(NCORE)))
    acc = np.zeros((D, B * S), dtype=np.float32)
    for c in range(NCORE):
        acc += res.results[c]["outT"].astype(np.float32)
    return np.ascontiguousarray(acc.T).reshape(B, S, D)


def kernel_traced(x, W_qkv, W_proj, trace=False):
    """Dev helper: also returns the BassKernelResults (exec_time_ns etc.)."""
    from concourse.bass_utils import run_bass_kernel_spmd

    in_maps = host_prepare(np.asarray(x), np.asarray(W_qkv), np.asarray(W_proj))
    nc = build()
    res = run_bass_kernel_spmd(nc, in_maps, list(range(NCORE)), trace=trace)
    acc = np.zeros((D, B * S), dtype=np.float32)
    for c in range(NCORE):
        acc += res.results[c]["outT"].astype(np.float32)
    out = np.ascontiguousarray(acc.T).reshape(B, S, D)
    return out, res


# revision 16
# speedup vs baseline: 1.1844x; 1.1844x over previous
"""Causal self-attention TRN2 kernel: build + host glue. (v6.2)

Sharding: tensor-parallel over heads. 16 heads / 8 cores = 2 heads per core.
Each core computes q/k/v for its 2 heads over all 4x2048 tokens, runs causal
attention, and produces a partial output projection outT [1024, 8192] (fp16)
(wp rows for its heads only). Host sums the 8 partials and transposes.

v6 changes over v5 (322us):
- QKV projections in fp8 (e4m3) with DoubleRow perf mode: 256-deep
  contraction per pass -> 4 matmuls per projection per token tile (was 8).
  Host scales W by 32 so fp8 weights have unit variance; the 1/32 is folded
  into the rope cos/sin tables (q,k) and into W_proj (v path).
- AV matmul in fp8 DoubleRow over key-tile PAIRS (256 keys per pass): exp
  output is written as fp8e4m3 (bias -2.25 inside the exp keeps values in
  [0, ~90] clear of e4m3 saturation at 240; the uniform e^-2.25 cancels in
  the softmax normalize). The ones-augmented V columns still produce the
  softmax denominators for free in PSUM rows 64-127.
- Causal masking moved off DVE onto the PE: one accumulation matmul
  (-1e4 * I stationary, precomputed [128,2,256] mask pattern moving) adds
  -1e4 to all masked score entries before the EXP.
- Early tokens (first 128 queries of each batch) have concentrated softmax
  weights, so fp8 elementwise noise passes through unsuppressed and blows
  the max-err metric. A small full-fp16 sidecar recomputes those 4x128
  queries exactly (fp16 QKV + rope + scores + AV for keys 0-127) and
  overwrites yTn[:, 0:128] before the output projection.
- AV tail + softmax normalize of each q-tile are deferred into the next
  q-tile's filler stream so the PE never waits on the final EXPs at a
  q-tile boundary (HAM clock-gate oscillation was costing ~45us at half
  clock). Warm matmuls are interleaved into the kernel tail for the same
  reason.
"""

from collections import deque
from contextlib import ExitStack

import numpy as np

import concourse.bacc as bacc
import concourse.bass as bass
import concourse.mybir as mybir
import concourse.tile as tile

F32 = mybir.dt.float32
FP16 = mybir.dt.float16
FP8 = mybir.dt.float8e4
AF = mybir.ActivationFunctionType
DR = mybir.MatmulPerfMode.DoubleRow

D = 1024
H = 16
DH = 64
S = 2048
B = 4
NCORE = 8
HPC = 2  # heads per core
NT = S // 512  # 4 token tiles per batch
NKT = S // 128  # 16 key tiles per batch
NPAIR = NKT // 2  # 8 key-tile pairs per batch

SWAP_MASK = [(i + 16) % 32 for i in range(32)]
EXP_BIAS = -2.25
WSCALE = 32.0
import os
DEFER_TAIL = os.environ.get("K_DEFER_TAIL", "1") == "1"
TAIL_WARM = os.environ.get("K_TAIL_WARM", "1") == "1"
SIDECAR = os.environ.get("K_SIDECAR", "1") == "1"
MERGED_TP = os.environ.get("K_MERGED_TP", "1") == "1"


def build(nb=B):
    nc = bacc.Bacc("TRN2")
    xT = nc.dram_tensor("xT", [D, B * S], FP8, kind="ExternalInput")
    wq = nc.dram_tensor("wq", [D, 128], FP8, kind="ExternalInput")
    wk = nc.dram_tensor("wk", [D, 128], FP8, kind="ExternalInput")
    wv = nc.dram_tensor("wv", [D, 128], FP8, kind="ExternalInput")
    wq16 = nc.dram_tensor("wq16", [D, 128], FP16, kind="ExternalInput")
    wk16 = nc.dram_tensor("wk16", [D, 128], FP16, kind="ExternalInput")
    wv16 = nc.dram_tensor("wv16", [D, 128], FP16, kind="ExternalInput")
    x16h = nc.dram_tensor("x16h", [D, B * 128], FP16, kind="ExternalInput")
    wp = nc.dram_tensor("wp", [128, D], FP16, kind="ExternalInput")
    cs1 = nc.dram_tensor("cs1", [128, S], FP16, kind="ExternalInput")
    cs2 = nc.dram_tensor("cs2", [128, S], FP16, kind="ExternalInput")
    cs1u = nc.dram_tensor("cs1u", [128, B * 128], FP16, kind="ExternalInput")
    cs2u = nc.dram_tensor("cs2u", [128, B * 128], FP16, kind="ExternalInput")
    maskC = nc.dram_tensor("maskC", [128, 512], FP16, kind="ExternalInput")
    negI = nc.dram_tensor("negI", [128, 128], FP16, kind="ExternalInput")
    ident = nc.dram_tensor("ident", [128, 128], FP16, kind="ExternalInput")
    outT = nc.dram_tensor("outT", [D, B * S], FP16, kind="ExternalOutput")
    warm = nc.dram_tensor("warm", [128, 128], FP16, kind="ExternalOutput")

    with tile.TileContext(nc) as tc, ExitStack() as ctx, nc.allow_low_precision(
        reason="fp8 matmul operands with fp32 accumulation; adequate accuracy"
    ):
        const = ctx.enter_context(tc.tile_pool(name="const", bufs=1))
        xt_pool = ctx.enter_context(tc.tile_pool(name="xt", bufs=3))
        slab = ctx.enter_context(tc.tile_pool(name="slab", bufs=3))
        tmp_pool = ctx.enter_context(tc.tile_pool(name="tmp", bufs=3))
        ex_pool = ctx.enter_context(tc.tile_pool(name="ex", bufs=5))
        ytn_pool = ctx.enter_context(tc.tile_pool(name="ytn", bufs=3))
        ob_pool = ctx.enter_context(tc.tile_pool(name="ob", bufs=4))
        # PSUM (8 banks): sc 2x[128,2,512]f32=4, yt 2x[128,512]f32=2, qp 2x=2
        ps_sc = ctx.enter_context(tc.tile_pool(name="ps_sc", bufs=2, space="PSUM"))
        ps_qp = ctx.enter_context(tc.tile_pool(name="ps_qp", bufs=2, space="PSUM"))
        ps_yt = ctx.enter_context(tc.tile_pool(name="ps_yt", bufs=2, space="PSUM"))

        # ---- constants (ordered so the first tile's work can start asap) ----
        id_sb = const.tile([128, 128], FP16)
        nc.sync.dma_start(out=id_sb[:], in_=ident[:])
        wq_sb = const.tile([128, 8, 128], FP8)
        nc.sync.dma_start(
            out=wq_sb[:, 0:4, :], in_=wq[:].rearrange("(a p) c -> p a c", p=128)[:, 0:4]
        )
        nc.sync.dma_start(
            out=wq_sb[:, 4:8, :], in_=wq[:].rearrange("(a p) c -> p a c", p=128)[:, 4:8]
        )
        xt0 = xt_pool.tile([128, 8, 512], FP8, tag="xt", name="xt0")
        for dt in range(8):
            nc.sync.dma_start(out=xt0[:, dt, :], in_=xT[bass.ts(dt, 128), 0:512])
        wk_sb = const.tile([128, 8, 128], FP8)
        wv_sb = const.tile([128, 8, 128], FP8)
        for w_sb, w_dram in ((wk_sb, wk), (wv_sb, wv)):
            nc.sync.dma_start(
                out=w_sb[:, 0:4, :],
                in_=w_dram[:].rearrange("(a p) c -> p a c", p=128)[:, 0:4],
            )
            nc.sync.dma_start(
                out=w_sb[:, 4:8, :],
                in_=w_dram[:].rearrange("(a p) c -> p a c", p=128)[:, 4:8],
            )
        cs1_sb = const.tile([128, S], FP16)
        cs2_sb = const.tile([128, S], FP16)
        nc.sync.dma_start(out=cs1_sb[:, 0:1024], in_=cs1[:, 0:1024])
        nc.sync.dma_start(out=cs2_sb[:, 0:1024], in_=cs2[:, 0:1024])
        mask_sb = const.tile([128, 2, 256], FP16)
        nc.sync.dma_start(
            out=mask_sb[:], in_=maskC[:].rearrange("p (a c) -> p a c", c=256)
        )
        negI_sb = const.tile([128, 128], FP16)
        nc.sync.dma_start(out=negI_sb[:], in_=negI[:])
        wp_sb = const.tile([128, D], FP16)
        nc.sync.dma_start(out=wp_sb[:], in_=wp[:])
        nc.sync.dma_start(out=cs1_sb[:, 1024:2048], in_=cs1[:, 1024:2048])
        nc.sync.dma_start(out=cs2_sb[:, 1024:2048], in_=cs2[:, 1024:2048])
        # fp16 sidecar constants (first 128 tokens of each batch)
        w16_sb = {}
        for nm, w_dram in (("q", wq16), ("k", wk16), ("v", wv16)):
            w16 = const.tile([128, 8, 128], FP16)
            nc.sync.dma_start(
                out=w16[:], in_=w_dram[:].rearrange("(a p) c -> p a c", p=128)
            )
            w16_sb[nm] = w16
        x16_sb = const.tile([128, 8, 512], FP16)
        for dt in range(8):
            nc.sync.dma_start(out=x16_sb[:, dt, :], in_=x16h[bass.ts(dt, 128), :])
        cs1u_sb = const.tile([128, 512], FP16)
        cs2u_sb = const.tile([128, 512], FP16)
        nc.sync.dma_start(out=cs1u_sb[:], in_=cs1u[:])
        nc.sync.dma_start(out=cs2u_sb[:], in_=cs2u[:])

        bias_sb = const.tile([128, 1], F32)
        nc.vector.memset(bias_sb[:], EXP_BIAS)
        # precise yTn for queries 0-127 of each batch, written by the
        # sidecar, copied over yTn at each batch's j=0 normalize
        ytn0 = const.tile([128, B * 128], FP16)
        v0aug = const.tile([128, B, HPC, 128], FP16)
        nc.vector.memset(v0aug[:, :, :, 64:128], 1.0)

        warm_ps = ps_qp.tile([128, 128], F32, tag="qp", name="warm_ps")
        for i in range(40):
            nc.tensor.matmul(warm_ps[:], id_sb[:], id_sb[:],
                             start=True, stop=True)
        warm_sb = tmp_pool.tile([128, 128], FP16, tag="t1", name="warm_sb")
        nc.vector.tensor_copy(warm_sb[:], warm_ps[:])
        nc.sync.dma_start(out=warm[:], in_=warm_sb[:])

        # pre-initialize the ones columns of all three v_sb rotation buffers
        # (only cols 0:64 of each [128,128] block are rewritten per batch)
        vsb_bufs = []
        for i in range(3):
            v_sb = slab.tile([128, HPC * NKT * 128], FP8, tag="v_sb", name="v_sb")
            v3d = v_sb[:].rearrange("p (n c) -> p n c", c=128)
            nc.vector.memset(v3d[:, 0 : HPC * NKT, 64:128], 1.0)
            vsb_bufs.append(v_sb)

        def rope16(dst_slice, qs, cs1_t, cs2_t, w=512):
            """dst(fp16) = qs*cs1_t + shuffle(qs)*cs2_t ; qs is [128,w] fp16."""
            sw = tmp_pool.tile([128, w], FP16, tag="sw", name="sw")
            nc.vector.stream_shuffle(sw[:], qs[:], mask=SWAP_MASK)
            t1 = tmp_pool.tile([128, w], FP16, tag="t1", name="t1")
            nc.vector.tensor_mul(t1[:], qs[:], cs1_t)
            t2 = tmp_pool.tile([128, w], FP16, tag="t2", name="t2")
            nc.vector.tensor_mul(t2[:], sw[:], cs2_t)
            nc.vector.tensor_add(dst_slice, t1[:], t2[:])

        # ---- fp16 sidecar: exact attention for queries 0-127 of each batch
        # (keys 0-127 only, causal). Emitted as early filler; writes ytn0.
        def sidecar_chunks():
            st8 = {}

            def mm8(ps, w16, n0, n1):
                for i in range(8):
                    nc.tensor.matmul(
                        ps[:], w16[:, i, :], x16_sb[:, i, :],
                        start=i == 0, stop=i == 7,
                    )

            def s_q():
                q_ps = ps_qp.tile([128, 512], F32, tag="qp", name="q0_ps")
                mm8(q_ps, w16_sb["q"], 0, 8)
                st8["q_ps"] = q_ps

            def s_k():
                q0s = tmp_pool.tile([128, 512], FP16, tag="qs", name="q0s")
                nc.scalar.copy(q0s[:], st8["q_ps"][:])
                st8["q0s"] = q0s
                k_ps = ps_qp.tile([128, 512], F32, tag="qp", name="k0_ps")
                mm8(k_ps, w16_sb["k"], 0, 8)
                st8["k_ps"] = k_ps

            def s_v():
                k0s = tmp_pool.tile([128, 512], FP16, tag="ks", name="k0s")
                nc.scalar.copy(k0s[:], st8["k_ps"][:])
                st8["k0s"] = k0s
                q0r = const.tile([128, 512], FP16)
                rope16(q0r[:], st8["q0s"], cs1u_sb[:], cs2u_sb[:])
                st8["q0r"] = q0r
                v_ps = ps_qp.tile([128, 512], F32, tag="qp", name="v0_ps")
                mm8(v_ps, w16_sb["v"], 0, 8)
                st8["v_ps"] = v_ps

            def s_tp():
                v0st = tmp_pool.tile([128, 512], FP16, tag="vst", name="v0st")
                nc.scalar.copy(v0st[:], st8["v_ps"][:])
                k0r = const.tile([128, 512], FP16)
                rope16(k0r[:], st8["k0s"], cs1u_sb[:], cs2u_sb[:])
                st8["k0r"] = k0r
                v0T = ps_qp.tile([128, 512], FP16, tag="qp", name="v0T")
                for c in range(4):
                    nc.tensor.transpose(
                        v0T[:, bass.ts(c, 128)],
                        v0st[:, bass.ts(c, 128)],
                        id_sb[:],
                    )
                v4 = v0T[:].rearrange("p (c hd) -> p c hd", hd=128)
                for h in range(HPC):
                    nc.vector.tensor_copy(
                        v0aug[:, :, h, 0:64],
                        v4[:, :, h * 64 : h * 64 + 64],
                    )

            def s_att(c):
                def run():
                    # NB: one PSUM bank cannot host concurrent accumulation
                    # groups with different PE tile positions (head A rows
                    # 0-63 vs head B rows 64-127) -> per-head tiles.
                    q0r, k0r = st8["q0r"], st8["k0r"]
                    sch = {}
                    for h in range(HPC):
                        sch[h] = ps_qp.tile([128, 512], F32, tag="qp",
                                            name=f"sc0{h}")
                    for h in range(HPC):
                        nc.tensor.matmul(
                            sch[h][:, 0:128],
                            k0r[bass.ts(h, 64), bass.ts(c, 128)],
                            q0r[bass.ts(h, 64), bass.ts(c, 128)],
                            start=True, stop=True,
                        )
                    for h in range(HPC):
                        nc.tensor.matmul(
                            sch[h][:, 0:128], negI_sb[:], mask_sb[:, 0, 0:128],
                            start=False, stop=True, skip_group_check=True,
                        )
                    ex0 = tmp_pool.tile([128, 2, 128], FP16, tag="ex0", name="ex0")
                    for h in range(HPC):
                        nc.scalar.activation(
                            ex0[:, h, :], sch[h][:, 0:128], AF.Exp,
                            scale=0.125, bias=bias_sb[:],
                        )
                    for h in range(HPC):
                        yt0 = ps_qp.tile([128, 512], F32, tag="qp", name="yt0")
                        nc.tensor.matmul(
                            yt0[:, 0:128], v0aug[:, c, h, :], ex0[:, h, :],
                            start=True, stop=True,
                        )
                        den = tmp_pool.tile([64, 128], F32, tag="den", name="den0")
                        nc.scalar.copy(den[:], yt0[64:128, 0:128])
                        rc = tmp_pool.tile([64, 128], F32, tag="rc64", name="rc0")
                        nc.vector.reciprocal_approx_fast(rc[:], den[:])
                        nc.vector.tensor_mul(
                            ytn0[bass.ts(h, 64), bass.ts(c, 128)],
                            yt0[0:64, 0:128],
                            rc[:],
                        )

                return run

            return [s_q, s_k, s_v, s_tp] + [s_att(c) for c in range(4)]

        # batch-generation state (slab tiles rotate per batch)
        cur = {}

        def p1_chunks(b, t, prefetch):
            """Return list of emission closures for token tile t of batch b.

            prefetch: (b', t') of the NEXT token tile, or None; its x DMA is
            emitted inside the first chunk.
            """

            def c_start():
                if t == 0:
                    qT = slab.tile([128, S], FP16, tag="qT", name="qT")
                    kT = slab.tile([128, S], FP16, tag="kT", name="kT")
                    v_sb = vsb_bufs[b % 3]
                    cur[b] = (qT, kT, v_sb)
                if prefetch is not None:
                    pb, pt = prefetch
                    xt_n = xt_pool.tile([128, 8, 512], FP8, tag="xt", name="xt_n")
                    for dt in range(8):
                        nc.sync.dma_start(
                            out=xt_n[:, dt, :],
                            in_=xT[
                                bass.ts(dt, 128),
                                pb * S + pt * 512 : pb * S + (pt + 1) * 512,
                            ],
                        )
                    cur["xt", pb, pt] = xt_n

            state = {}

            def get_xt():
                return cur.pop(("xt", b, t)) if ("xt", b, t) in cur else xt0

            def dr_proj(ps, w_sb, x):
                for i in range(4):
                    nc.tensor.matmul(
                        ps[:], w_sb[:, 2 * i : 2 * i + 2, :],
                        x[:, 2 * i : 2 * i + 2, :],
                        start=i == 0, stop=i == 3, perf_mode=DR,
                    )

            def c_q():
                x = state.setdefault("xt", get_xt())
                q_ps = ps_qp.tile([128, 512], F32, tag="qp", name="q_ps")
                state["q_ps"] = q_ps
                dr_proj(q_ps, wq_sb, x)

            def c_k():
                qs = tmp_pool.tile([128, 512], FP16, tag="qs", name="qs")
                nc.scalar.copy(qs[:], state["q_ps"][:])
                state["qs"] = qs
                x = state["xt"]
                k_ps = ps_qp.tile([128, 512], F32, tag="qp", name="k_ps")
                state["k_ps"] = k_ps
                dr_proj(k_ps, wk_sb, x)

            def c_v():
                ks = tmp_pool.tile([128, 512], FP16, tag="ks", name="ks")
                nc.scalar.copy(ks[:], state["k_ps"][:])
                state["ks"] = ks
                rope16(
                    cur[b][0][:, bass.ts(t, 512)], state["qs"],
                    cs1_sb[:, bass.ts(t, 512)], cs2_sb[:, bass.ts(t, 512)],
                )
                x = state["xt"]
                v_ps = ps_qp.tile([128, 512], F32, tag="qp", name="v_ps")
                state["v_ps"] = v_ps
                dr_proj(v_ps, wv_sb, x)

            def c_vev():
                vstage = tmp_pool.tile([128, 512], FP16, tag="vst", name="vstage")
                nc.vector.tensor_copy(vstage[:], state["v_ps"][:])
                state["vst"] = vstage
                rope16(
                    cur[b][1][:, bass.ts(t, 512)], state["ks"],
                    cs1_sb[:, bass.ts(t, 512)], cs2_sb[:, bass.ts(t, 512)],
                )

            def c_tp():
                # transpose both heads at once: [128,128] blocks of vstage
                # -> [tok, 2*64 dims]; scatter per head into v_sb (fp8)
                vstage = state["vst"]
                tpb = ps_qp.tile([128, 512], FP16, tag="qp", name="tpb")
                for kk in range(4):
                    nc.tensor.transpose(
                        tpb[:, bass.ts(kk, 128)],
                        vstage[:, bass.ts(kk, 128)],
                        id_sb[:],
                    )
                v_sb = cur[b][2]
                t4 = tpb[:].rearrange("p (n hd) -> p n hd", hd=128)
                for h in range(HPC):
                    dst = v_sb[:].rearrange("p (n c) -> p n c", c=128)[
                        :, h * NKT + t * 4 : h * NKT + t * 4 + 4, 0:64
                    ]
                    nc.vector.tensor_copy(dst, t4[:, :, h * 64 : h * 64 + 64])

            def c_tp_v5(h):
                def run():
                    vstage = state["vst"]
                    tp4 = ps_qp.tile([128, 256], FP16, tag="qp", name="tp4")
                    for kk in range(4):
                        nc.tensor.transpose(
                            tp4[:, bass.ts(kk, 64)],
                            vstage[bass.ts(h, 64), bass.ts(kk, 128)],
                            id_sb[bass.ts(h, 64), bass.ts(h, 64)],
                        )
                    v_sb = cur[b][2]
                    dst = v_sb[:].rearrange("p (n c) -> p n c", c=128)[
                        :, h * NKT + t * 4 : h * NKT + t * 4 + 4, 0:64
                    ]
                    nc.vector.tensor_copy(
                        dst, tp4[:].rearrange("p (n c) -> p n c", c=64)
                    )

                return run

            if MERGED_TP:
                return [c_start, c_q, c_k, c_v, c_vev, c_tp]
            return [c_start, c_q, c_k, c_v, c_vev, c_tp_v5(0), c_tp_v5(1)]

        def att_emit(b, j, fill, tail=False):
            """Emit attention for qtile j of batch b, draining `fill` units
            (independent PE work) evenly across the key-pair slots. The AV
            tail, softmax normalize, and output projection are returned as
            filler for the NEXT q-tile."""
            qT, kT, v_sb = cur[b]
            tok0 = b * S
            npair = 2 * (j + 1)
            n_fill = len(fill)
            popped = 0
            yts = {}
            for h in range(HPC):
                yts[h] = ps_yt.tile([128, 512], F32, tag="yt", name=f"yt{h}")
            exs = {}
            starts = {}
            v4d = v_sb[:].rearrange("p (n c) -> p n c", c=128)

            def av_p(p, last):
                st = starts[p]
                for h in range(HPC):
                    n0 = h * NKT + 2 * p
                    nc.tensor.matmul(
                        yts[h][:, st:512],
                        v4d[:, n0 : n0 + 2, :],
                        exs[p, h][:, :, st:512],
                        start=(p == 0),
                        stop=last,
                        perf_mode=DR,
                    )

            for p in range(npair):
                st = 256 if p == 2 * j + 1 else 0
                diag = p >= 2 * j
                starts[p] = st
                scs = {}
                for h in range(HPC):
                    # head A: PE rows 0-63, head B: rows 64-127 — emitted
                    # back-to-back so the K=64 row tiles run concurrently
                    scs[h] = ps_sc.tile([128, 2, 512], F32, tag="sc", name=f"sc{h}")
                for sl in range(2):
                    for h in range(HPC):
                        kt = 2 * p + sl
                        nc.tensor.matmul(
                            scs[h][:, sl, st:512],
                            kT[bass.ts(h, 64), bass.ts(kt, 128)],
                            qT[bass.ts(h, 64), j * 512 + st : (j + 1) * 512],
                            start=True,
                            stop=True,
                        )
                if diag:
                    # add -1e4 to masked (non-causal) entries of the two
                    # diagonal key tiles before the exp
                    for h in range(HPC):
                        nc.tensor.matmul(
                            scs[h][:, :, st : st + 256],
                            negI_sb[:],
                            mask_sb[:],
                            start=False,
                            stop=True,
                            skip_group_check=True,
                        )
                for h in range(HPC):
                    ex = ex_pool.tile([128, 2, 512], FP8, tag="ex", name="ex")
                    nc.scalar.activation(
                        ex[:, :, st:512], scs[h][:, :, st:512], AF.Exp,
                        scale=0.125, bias=bias_sb[:],
                    )
                    exs[p, h] = ex
                if p > 0:
                    av_p(p - 1, last=False)
                # drain filler units evenly
                want = (n_fill * (p + 1)) // (npair + 1)
                while popped < want and fill:
                    fill.popleft()()
                    popped += 1
            if not DEFER_TAIL:
                av_p(npair - 1, last=True)
            while fill and popped < n_fill:
                fill.popleft()()
                popped += 1

            yTn = ytn_pool.tile([128, 512], FP16, tag="ytn", name="yTn")

            def av_tail():
                if DEFER_TAIL:
                    av_p(npair - 1, last=True)

            def norm_chunk():
                for h in range(HPC):
                    # yt rows 64..127 hold the denominator (ones columns of
                    # v_aug); staged via DVE tensor_copy (offset-64 PSUM
                    # reads verified OK for tensor_copy) so the ACT queue
                    # stays free for EXPs.
                    den = tmp_pool.tile([64, 512], F32, tag="den", name="den")
                    nc.vector.tensor_copy(den[:], yts[h][64:128, :])
                    rc64 = tmp_pool.tile([64, 512], F32, tag="rc64", name="rc64")
                    nc.vector.reciprocal_approx_fast(rc64[:], den[:])
                    nc.vector.tensor_mul(
                        yTn[bass.ts(h, 64), :], yts[h][0:64, :], rc64[:]
                    )
                if j == 0 and SIDECAR:
                    # overwrite the concentrated-softmax early queries with
                    # the fp16 sidecar's exact values
                    nc.vector.tensor_copy(
                        yTn[:, 0:128], ytn0[:, bass.ts(b, 128)]
                    )

            def proj_chunk(dt):
                def run():
                    po = ps_qp.tile([128, 512], F32, tag="qp", name="po")
                    nc.tensor.matmul(
                        po[:], wp_sb[:, bass.ts(dt, 128)], yTn[:],
                        start=True, stop=True,
                    )
                    ob = ob_pool.tile([128, 512], FP16, tag="ob", name="ob")
                    nc.vector.tensor_copy(ob[:], po[:])
                    nc.sync.dma_start(
                        out=outT[
                            bass.ts(dt, 128), tok0 + j * 512 : tok0 + (j + 1) * 512
                        ],
                        in_=ob[:],
                    )
                    if tail and TAIL_WARM:
                        # keep the PE dense through the drain so the HAM
                        # clock gate stays at full rate
                        wp2 = ps_qp.tile([128, 512], F32, tag="qp", name="wm")
                        nc.tensor.matmul(
                            wp2[:], id_sb[:], cs1_sb[:, 0:512],
                            start=True, stop=True,
                        )

                return run

            return [av_tail, norm_chunk] + [proj_chunk(dt) for dt in range(8)]

        # ---- driver: p1 units run 2 steps ahead of att units ----
        p1s = [(b, t) for b in range(nb) for t in range(NT)]
        atts = [(b, j) for b in range(nb) for j in range(NT)]
        fill = deque()
        for i in range(len(p1s) + 2):
            if i < len(p1s):
                pref = p1s[i + 1] if i + 1 < len(p1s) else None
                fill.extend(p1_chunks(*p1s[i], prefetch=pref))
                if i == 0 and SIDECAR:
                    fill.extend(sidecar_chunks())
            if i >= 2:
                proj = att_emit(*atts[i - 2], fill=fill, tail=(i - 2) == len(atts) - 1)
                fill.extend(proj)
            elif i < 2:
                while fill:
                    fill.popleft()()
        while fill:
            fill.popleft()()
    nc.finalize()
    return nc


# ---------------- host side ----------------

def host_prepare(x, W_qkv, W_proj):
    import ml_dtypes

    fp8 = ml_dtypes.float8_e4m3

    def to_fp8(a):
        return np.clip(a, -240.0, 240.0).astype(fp8)

    xf = np.ascontiguousarray(np.asarray(x, dtype=np.float32).reshape(B * S, D))
    xT = np.ascontiguousarray(to_fp8(xf.T))
    # fp16 copy of the first 128 tokens of each batch for the sidecar
    x16h = np.ascontiguousarray(
        np.concatenate([xf[b * S : b * S + 128] for b in range(B)], axis=0).T
    ).astype(np.float16)
    Wq = np.asarray(W_qkv[:, 0:D], dtype=np.float32)
    Wk = np.asarray(W_qkv[:, D : 2 * D], dtype=np.float32)
    Wv = np.asarray(W_qkv[:, 2 * D : 3 * D], dtype=np.float32)
    Wp = np.asarray(W_proj, dtype=np.float32)
    half = DH // 2
    inv_freq = 1.0 / (10000.0 ** (np.arange(half, dtype=np.float64) / half))
    freqs = np.outer(np.arange(S, dtype=np.float64), inv_freq)  # [S, 32]
    cos = np.cos(freqs)
    sin = np.sin(freqs)
    # quadrant-local rope pair layout: per 32-slot quadrant q, slots 0-15
    # hold even dims of pairs 16q..16q+15, slots 16-31 the odd dims.
    perm = np.empty(DH, dtype=np.int64)
    cs1_h = np.empty((DH, S), dtype=np.float32)
    cs2_h = np.empty((DH, S), dtype=np.float32)
    for q in range(2):
        for i in range(32):
            k = 16 * q + (i % 16)
            r = 32 * q + i
            perm[r] = 2 * k if i < 16 else 2 * k + 1
            cs1_h[r] = cos[:, k]
            cs2_h[r] = -sin[:, k] if i < 16 else sin[:, k]
    # fold the 1/WSCALE compensation for the fp8 weight scaling into rope
    cs1 = (np.concatenate([cs1_h, cs1_h], axis=0) / WSCALE).astype(np.float16)
    cs2 = (np.concatenate([cs2_h, cs2_h], axis=0) / WSCALE).astype(np.float16)
    # unscaled rope tables for positions 0-127, repeated per batch
    cs1u = np.ascontiguousarray(
        np.tile(np.concatenate([cs1_h, cs1_h], axis=0)[:, 0:128], (1, B))
    ).astype(np.float16)
    cs2u = np.ascontiguousarray(
        np.tile(np.concatenate([cs2_h, cs2_h], axis=0)[:, 0:128], (1, B))
    ).astype(np.float16)
    ident = np.eye(128, dtype=np.float16)
    negI = (-1.0e4 * np.eye(128)).astype(np.float16)
    # mask pattern for a diagonal key-tile pair: moving operand [128, 2, 256]
    ii = np.arange(128)[:, None]
    jj = np.arange(128)[None, :]
    low = (ii > jj).astype(np.float16)  # strictly-lower = non-causal
    maskC = np.concatenate(
        [low, np.zeros((128, 128), np.float16),
         np.ones((128, 128), np.float16), low],
        axis=1,
    )
    in_maps = []
    for c in range(NCORE):
        hA, hB = HPC * c, HPC * c + 1

        def cols(W, h, p=None):
            w = W[:, h * DH : (h + 1) * DH]
            return w[:, p] if p is not None else w

        wq_c = np.concatenate([cols(Wq, hA, perm), cols(Wq, hB, perm)], axis=1)
        wk_c = np.concatenate([cols(Wk, hA, perm), cols(Wk, hB, perm)], axis=1)
        wv_c = np.concatenate([cols(Wv, hA), cols(Wv, hB)], axis=1)
        in_maps.append(
            {
                "xT": xT,
                "x16h": x16h,
                "wq": to_fp8(WSCALE * wq_c),
                "wk": to_fp8(WSCALE * wk_c),
                "wv": to_fp8(WSCALE * wv_c),
                "wq16": wq_c.astype(np.float16),
                "wk16": wk_c.astype(np.float16),
                "wv16": (wv_c * WSCALE).astype(np.float16),
                "wp": np.ascontiguousarray(
                    Wp[hA * DH : (hB + 1) * DH, :] / WSCALE
                ).astype(np.float16),
                "cs1": cs1,
                "cs2": cs2,
                "cs1u": cs1u,
                "cs2u": cs2u,
                "maskC": maskC,
                "negI": negI,
                "ident": ident,
            }
        )
    return in_maps


def kernel(x, W_qkv, W_proj):
    """Grading entrypoint: full inputs in, full output out.

    x [4, 2048, 1024] fp32, W_qkv [1024, 3072] fp32, W_proj [1024, 1024] fp32
    -> [4, 2048, 1024] fp32
    """
    from concourse.bass_utils import run_bass_kernel_spmd

    x = np.asarray(x)
    in_maps = host_prepare(x, np.asarray(W_qkv), np.asarray(W_proj))
    nc = build()
    res = run_bass_kernel_spmd(nc, in_maps, list(range(NCORE)))
    acc = np.zeros((D, B * S), dtype=np.float32)
    for c in range(NCORE):
        acc += res.results[c]["outT"].astype(np.float32)
    return np.ascontiguousarray(acc.T).reshape(B, S, D)


def kernel_traced(x, W_qkv, W_proj, trace=False):
    """Dev helper: also returns the BassKernelResults (exec_time_ns etc.)."""
    from concourse.bass_utils import run_bass_kernel_spmd

    in_maps = host_prepare(np.asarray(x), np.asarray(W_qkv), np.asarray(W_proj))
    nc = build()
    res = run_bass_kernel_spmd(nc, in_maps, list(range(NCORE)), trace=trace)
    acc = np.zeros((D, B * S), dtype=np.float32)
    for c in range(NCORE):
        acc += res.results[c]["outT"].astype(np.float32)
    out = np.ascontiguousarray(acc.T).reshape(B, S, D)
    return out, res
